# revision 23
# baseline (speedup 1.0000x reference)
"""Trainium2 Bass kernel for nn_ChaPromptGenBlock (moe_routing).

Strategy: data-parallel over H across 8 cores (24 H-rows each, 9216 tokens).
- emb = mean(x) needs a global reduce -> on-device AllReduce of [128,2] partials.
- gating logits computed token-major in full fp32 (top-2 index sets feed the
  aux-loss counts, so they must match the reference's selections exactly).
- dense 4-expert MLP in bf16, column-major tokens; fc2 accumulates all experts
  in PSUM with gates pre-multiplied into h; b2 enters as a K=4 matmul vs gates.
- residual y + x in fp32; importance/load partial-reduced on device, cv^2 loss
  finished on host in fp32.
"""
import sys

sys.path.insert(0, "/opt/trn_rl_repo")

import numpy as np
import ml_dtypes

import concourse.bass as bass
import concourse.mybir as mybir
import concourse.tile as tile
from concourse import library_config
from concourse.bass_utils import run_bass_kernel_spmd
from concourse.vector_clock import ScopedClock, VectorClock

# ---------------------------------------------------------------------------
# This walrus build caps sync-waits per instruction; Tile's exit drain piles
# every outstanding proc's wait onto one Drain.  Split it into a chain of
# single-wait drains (sequential on the SP engine, so semantics preserved).
def _patched_drain_and_barrier(self, tick_clock, wait_clock):
    gc = tick_clock.global_clock
    n = len(gc)
    procs = [p for p in range(n) if gc[p] > 0]
    for p in procs:
        vec = [gc[q] if q == p else 0 for q in range(n)]
        d = self.nc.sync.drain()
        wait_clock.add_sem_waits(d.ins, ScopedClock({None: VectorClock(vec)}))
    if not procs:
        self.nc.sync.drain()
    self.nc.all_engine_barrier()
    assert self.sems is not None
    popped = self.nc._tile_sem_poison_stack.pop()
    assert popped is self._sem_poison
    self.nc.clear_and_free_semaphores(list(self.sems.allocated().values()))
    self.nc.all_engine_barrier()


tile.TileContext._drain_and_barrier = _patched_drain_and_barrier


def _split_multi_waits(nc, limit=1):
    """Walrus here allows only `limit` sync-waits per instruction; move the
    excess onto same-engine NOPs inserted immediately before."""
    nid = 0
    for bb in nc.main_func.blocks:
        newlist = []
        for ins in bb.instructions:
            si = ins.sync_info
            if si is not None and len(si.on_wait) > limit:
                waits = list(si.on_wait)
                head, tail = waits[:-limit], waits[-limit:]
                for i in range(0, len(head), limit):
                    nid += 1
                    nop = mybir.InstNoOp(name=f"I-wsplit-{nid}", ins=[], outs=[])
                    nop.engine = ins.engine
                    nop.sync_info = mybir.SyncInfo(on_wait=head[i:i + limit],
                                                   on_update=[])
                    nc.register_instruction(nop)
                    newlist.append(nop)
                ins.sync_info = mybir.SyncInfo(on_wait=tail,
                                               on_update=list(si.on_update))
            newlist.append(ins)
        bb.instructions[:] = newlist
# ---------------------------------------------------------------------------

F32 = mybir.dt.float32
F32R = mybir.dt.float32r
BF16 = mybir.dt.bfloat16
AX = mybir.AxisListType
ALU = mybir.AluOpType
ACTF = mybir.ActivationFunctionType

N_CORES = 8
B = 2
C = 128
H = 192
W = 192
HS = H // N_CORES            # 24 H-rows per core
TB = HS * W                  # 4608 tokens per (core, batch)
MT = 1536                    # megatile tokens; TB = 3 * MT
NMT = TB // MT
NTC = MT // 128              # 12 token-chunks of 128 per megatile
E = 4
NP5 = 5
HID = 340
HID_E = 352                  # per-expert hidden padded to 32-multiple
HID_F = E * HID_E            # 1408
NCH = 11
HID_P = NCH * 128            # 1408
HW_N = H * W
BF16_NP = ml_dtypes.bfloat16

# expert segments inside each 128-row hidden chunk: (chunk, p0, p1, expert).
# engine partition-access rule: start 0 -> any count; start 64 -> <=64;
# start 32/96 -> <=32.  expert boundaries at multiples of 352 are 32-aligned.
def _legal_blocks(p0, p1):
    blocks = []
    p = p0
    while p < p1:
        if p == 0:
            q = p1
        elif p == 64:
            q = min(p1, 128)
        else:  # 32 or 96
            q = min(p1, p + 32)
        blocks.append((p, q))
        p = q
    return blocks


SEGMENTS = []
for kc in range(NCH):
    lo = kc * 128
    hi = lo + 128
    p = lo
    while p < hi:
        e = p // HID_E
        pe = min((e + 1) * HID_E, hi)
        for (bp, bq) in _legal_blocks(p - lo, pe - lo):
            SEGMENTS.append((kc, bp, bq, e))
        p = pe


def build_program(n_cores=N_CORES):
    nc = bass.Bass("TRN2", target_bir_lowering=False, debug=False,
                   num_devices=n_cores)

    x_d = nc.declare_dram_parameter("x", [B, C, TB], F32, isOutput=False)
    wconvT_d = nc.declare_dram_parameter("wconvT", [C, C], F32, isOutput=False)
    wgate_d = nc.declare_dram_parameter("wgate", [C, E], F32, isOutput=False)
    wlinT_d = nc.declare_dram_parameter("wlinT", [C, NP5], F32, isOutput=False)
    blin2_d = nc.declare_dram_parameter("blin2", [B, NP5], F32, isOutput=False)
    prompt_d = nc.declare_dram_parameter("prompt", [NP5, C], F32, isOutput=False)
    w1T_d = nc.declare_dram_parameter("w1T", [C, HID_P], BF16, isOutput=False)
    w2T_d = nc.declare_dram_parameter("w2T", [HID_P, C], BF16, isOutput=False)
    b1T_d = nc.declare_dram_parameter("b1T", [C, NCH], F32, isOutput=False)
    b2_d = nc.declare_dram_parameter("b2", [E, C], BF16, isOutput=False)
    identf_d = nc.declare_dram_parameter("identf", [C, C], F32, isOutput=False)
    identb_d = nc.declare_dram_parameter("identb", [C, C], BF16, isOutput=False)
    ones_d = nc.declare_dram_parameter("ones", [C, 1], F32, isOutput=False)

    y_d = nc.declare_dram_parameter("y", [B, C, TB], F32, isOutput=True)
    misc_d = nc.declare_dram_parameter("misc", [1, 8], F32, isOutput=True)

    with tile.TileContext(nc) as tc:
        with tc.tile_pool(name="const", bufs=1) as const, \
             tc.tile_pool(name="persist", bufs=1) as persist, \
             tc.tile_pool(name="dram", bufs=1, space="DRAM") as dram:

            # ---- constants into SBUF -------------------------------------
            wconvT = const.tile([C, C], F32)
            nc.sync.dma_start(out=wconvT, in_=wconvT_d[:])
            wgate = const.tile([C, E], F32)
            nc.sync.dma_start(out=wgate, in_=wgate_d[:])
            wlinT = const.tile([C, NP5], F32)
            nc.sync.dma_start(out=wlinT, in_=wlinT_d[:])
            blin2 = const.tile([B, NP5], F32)
            nc.sync.dma_start(out=blin2, in_=blin2_d[:])
            promptt = const.tile([NP5, C], F32)
            nc.sync.dma_start(out=promptt, in_=prompt_d[:])
            w1T = const.tile([C, HID_P], BF16)
            nc.sync.dma_start(out=w1T, in_=w1T_d[:])
            w2T = const.tile([C, NCH, C], BF16)
            nc.sync.dma_start(out=w2T, in_=w2T_d[:].rearrange("(k p) c -> p k c", p=128))
            b1T = const.tile([C, NCH], F32)
            nc.sync.dma_start(out=b1T, in_=b1T_d[:])
            b2t = const.tile([E, C], BF16)
            nc.sync.dma_start(out=b2t, in_=b2_d[:])
            identf = const.tile([C, C], F32)
            nc.sync.dma_start(out=identf, in_=identf_d[:])
            identb = const.tile([C, C], BF16)
            nc.sync.dma_start(out=identb, in_=identb_d[:])
            ones = const.tile([C, 1], F32)
            nc.sync.dma_start(out=ones, in_=ones_d[:])

            # ---- persistent working tensors ------------------------------
            xs = []
            for b in range(B):
                xb = persist.tile([C, TB], F32, tag=f"x{b}")
                for m in range(NMT):
                    nc.sync.dma_start(out=xb[:, m * MT:(m + 1) * MT],
                                      in_=x_d[b][:, m * MT:(m + 1) * MT])
                xs.append(xb)
            tys = [persist.tile([C, TB], BF16, tag=f"ty{b}", name=f"ty{b}") for b in range(B)]
            g_sb = persist.tile([E, B * TB], BF16, tag="g")
            accimp = persist.tile([C, E], F32, tag="accimp")
            accld = persist.tile([C, E], F32, tag="accld")
            nc.vector.memset(accimp, 0.0)
            nc.vector.memset(accld, 0.0)
            embs = persist.tile([C, B], F32, tag="embs")
            sconv = [persist.tile([C, C], F32, tag=f"sconv{b}", name=f"sconv{b}") for b in range(B)]

            # ---- emb partial + AllReduce ---------------------------------
            embp = persist.tile([C, B], F32, tag="embp")
            for b in range(B):
                nc.vector.reduce_sum(out=embp[:, b:b + 1], in_=xs[b][:, :], axis=AX.X)
            g_dram = dram.tile([E, B * TB], BF16)
            cc_in = dram.tile([C, B], F32)
            cc_out = dram.tile([C, B], F32,
                               addr_space="Shared" if n_cores > 4 else "Local")
            nc.sync.dma_start(out=cc_in[:], in_=embp[:])
            nc.gpsimd.collective_compute(
                "AllReduce", ALU.add,
                replica_groups=[list(range(n_cores))],
                ins=[cc_in[:]], outs=[cc_out[:]],
            )
            nc.sync.dma_start(out=embs[:], in_=cc_out[:])

            # ---- tiny chain: pw -> spectral prompt -> scaled conv wts ----
            with tc.tile_pool(name="tiny", bufs=1) as tiny, \
                 tc.tile_pool(name="tinyp", bufs=1, space="PSUM") as tinyp:
                psL1 = tinyp.tile([B, NP5], F32, tag="psL1")
                nc.tensor.matmul(psL1[:], embs[:], wlinT[:], start=True, stop=True)
                Ls = tiny.tile([B, NP5], F32, tag="Ls")
                nc.vector.tensor_add(Ls[:], psL1[:], blin2[:])
                mx = tiny.tile([B, 1], F32, tag="mx")
                nc.vector.reduce_max(out=mx[:], in_=Ls[:], axis=AX.X)
                nmx = tiny.tile([B, 1], F32, tag="nmx")
                nc.vector.tensor_scalar_mul(nmx[:], mx[:], -1.0)
                ex = tiny.tile([B, NP5], F32, tag="ex")
                nc.scalar.activation(out=ex[:], in_=Ls[:], func=ACTF.Exp,
                                     bias=nmx[:], scale=1.0)
                ssum = tiny.tile([B, 1], F32, tag="ssum")
                nc.vector.reduce_sum(out=ssum[:], in_=ex[:], axis=AX.X)
                rs = tiny.tile([B, 1], F32, tag="rs")
                nc.vector.reciprocal(rs[:], ssum[:])
                pw = tiny.tile([B, NP5], F32, tag="pw")
                nc.vector.tensor_scalar(pw[:], ex[:], rs[:], None, ALU.mult)
                psPwT = tinyp.tile([NP5, B], F32, tag="psPwT")
                nc.tensor.transpose(psPwT[:], pw[:], identf[:B, :B])
                pwT = tiny.tile([NP5, B], F32, tag="pwT")
                nc.vector.tensor_copy(pwT[:], psPwT[:])
                psS = tinyp.tile([C, B], F32, tag="psS")
                nc.tensor.matmul(psS[:], promptt[:], pwT[:], start=True, stop=True)
                s_sb = tiny.tile([C, B], F32, tag="s_sb")
                nc.vector.tensor_copy(s_sb[:], psS[:])
                for b in range(B):
                    nc.vector.tensor_scalar(sconv[b][:], wconvT[:],
                                            s_sb[:, b:b + 1], None, ALU.mult)

            # ======== era 1: gating + conv ================================
            with tc.tile_pool(name="psL", bufs=2, space="PSUM") as psL_pool, \
                 tc.tile_pool(name="psG", bufs=2, space="PSUM") as psG_pool, \
                 tc.tile_pool(name="psC", bufs=1, space="PSUM") as psC_pool, \
                 tc.tile_pool(name="top2", bufs=3) as top2:
                for b in range(B):
                    for m in range(NMT):
                        off = m * MT
                        goff = b * TB + off
                        psL = psL_pool.tile([C, NTC, E], F32, tag="psL")
                        for k in range(NTC):
                            nc.tensor.matmul(
                                psL[:, k, :],
                                xs[b][:, off + k * 128: off + (k + 1) * 128],
                                wgate[:], start=True, stop=True)
                        L = top2.tile([C, NTC, E], F32, tag="L")
                        nc.vector.tensor_copy(L[:], psL[:])
                        m1 = top2.tile([C, NTC], F32, tag="m1")
                        nc.vector.reduce_max(out=m1[:], in_=L[:], axis=AX.X)
                        m1b = m1[:].unsqueeze(2).broadcast_to([C, NTC, E])
                        eq1 = top2.tile([C, NTC, E], F32, tag="eq1")
                        nc.vector.tensor_tensor(eq1[:], L[:], m1b, ALU.is_equal)
                        Lm = top2.tile([C, NTC, E], F32, tag="Lm")
                        nc.vector.scalar_tensor_tensor(
                            Lm[:], eq1[:], -1e30, L[:], ALU.mult, ALU.add)
                        m2 = top2.tile([C, NTC], F32, tag="m2")
                        nc.vector.reduce_max(out=m2[:], in_=Lm[:], axis=AX.X)
                        m2b = m2[:].unsqueeze(2).broadcast_to([C, NTC, E])
                        eq2 = top2.tile([C, NTC, E], F32, tag="eq2")
                        nc.vector.tensor_tensor(eq2[:], L[:], m2b, ALU.is_equal)
                        d = top2.tile([C, NTC], F32, tag="d")
                        nc.vector.tensor_sub(d[:], m2[:], m1[:])
                        ed = top2.tile([C, NTC], F32, tag="ed")
                        nc.scalar.activation(out=ed[:], in_=d[:], func=ACTF.Exp)
                        den = top2.tile([C, NTC], F32, tag="den")
                        nc.vector.tensor_scalar_add(den[:], ed[:], 1.0)
                        w1g = top2.tile([C, NTC], F32, tag="w1g")
                        nc.vector.reciprocal(w1g[:], den[:])
                        w2g = top2.tile([C, NTC], F32, tag="w2g")
                        nc.vector.tensor_mul(w2g[:], ed[:], w1g[:])
                        w1gb = w1g[:].unsqueeze(2).broadcast_to([C, NTC, E])
                        w2gb = w2g[:].unsqueeze(2).broadcast_to([C, NTC, E])
                        g1 = top2.tile([C, NTC, E], F32, tag="g1")
                        nc.vector.tensor_tensor(g1[:], eq1[:], w1gb, ALU.mult)
                        g2 = top2.tile([C, NTC, E], F32, tag="g2")
                        nc.vector.tensor_tensor(g2[:], eq2[:], w2gb, ALU.mult)
                        gates = top2.tile([C, NTC, E], F32, tag="gates")
                        nc.vector.tensor_add(gates[:], g1[:], g2[:])
                        # importance / load partial sums
                        tmp4 = top2.tile([C, E], F32, tag="tmp4")
                        nc.vector.reduce_sum(out=tmp4[:],
                                             in_=gates[:].transpose([0, 2, 1]),
                                             axis=AX.X)
                        nc.vector.tensor_add(accimp[:], accimp[:], tmp4[:])
                        msk = top2.tile([C, NTC, E], F32, tag="msk")
                        nc.vector.tensor_add(msk[:], eq1[:], eq2[:])
                        tmp4b = top2.tile([C, E], F32, tag="tmp4b")
                        nc.vector.reduce_sum(out=tmp4b[:],
                                             in_=msk[:].transpose([0, 2, 1]),
                                             axis=AX.X)
                        nc.vector.tensor_add(accld[:], accld[:], tmp4b[:])
                        # gates -> bf16 -> transpose to [4, MT] layout
                        gb = top2.tile([C, NTC, E], BF16, tag="gb")
                        nc.vector.tensor_copy(gb[:], gates[:])
                        for q in range(NTC // 4):
                            psG = psG_pool.tile([E, 512], BF16, tag="psG")
                            for kk in range(4):
                                k = q * 4 + kk
                                nc.tensor.transpose(
                                    psG[:, kk * 128:(kk + 1) * 128],
                                    gb[:, k, :], identb[:])
                            nc.vector.tensor_copy(
                                g_sb[:, goff + q * 512: goff + (q + 1) * 512],
                                psG[:])
                            nc.sync.dma_start(
                                out=g_dram[:, goff + q * 512: goff + (q + 1) * 512],
                                in_=g_sb[:, goff + q * 512: goff + (q + 1) * 512])
                        # conv (after AllReduce chain -> sconv ready)
                        psCv = psC_pool.tile([C, MT], F32, tag="psCv")
                        for i in range(3):
                            nc.tensor.matmul(
                                psCv[:, i * 512:(i + 1) * 512],
                                sconv[b][:],
                                xs[b][:, off + i * 512: off + (i + 1) * 512],
                                start=True, stop=True)
                        nc.vector.tensor_copy(tys[b][:, off:off + MT], psCv[:])

            # ======== era 2: experts ======================================
            with tc.tile_pool(name="psH", bufs=2, space="PSUM") as psH_pool, \
                 tc.tile_pool(name="psY", bufs=2, space="PSUM") as psY_pool, \
                 tc.tile_pool(name="hp", bufs=2) as hpool, \
                 tc.tile_pool(name="Gp", bufs=2) as Gpool, \
                 tc.tile_pool(name="yp", bufs=2) as ypool:
                for b in range(B):
                    for m in range(NMT):
                        off = m * MT
                        goff = b * TB + off
                        Gs = []
                        for e in range(E):
                            Ge = Gpool.tile([C, MT], BF16, tag=f"G{e}")
                            nc.sync.dma_start(
                                out=Ge[:],
                                in_=g_dram[e:e + 1, goff:goff + MT]
                                .broadcast_to([C, MT]))
                            Gs.append(Ge)
                        hs = []
                        for kc in range(NCH):
                            psH = psH_pool.tile([C, MT], F32, tag="psH")
                            for i in range(3):
                                nc.tensor.matmul(
                                    psH[:, i * 512:(i + 1) * 512],
                                    w1T[:, kc * 128:(kc + 1) * 128],
                                    tys[b][:, off + i * 512: off + (i + 1) * 512],
                                    start=True, stop=True)
                            hk = hpool.tile([C, MT], BF16, tag=f"h{kc}")
                            nc.scalar.activation(out=hk[:], in_=psH[:],
                                                 func=ACTF.Gelu,
                                                 bias=b1T[:, kc:kc + 1], scale=1.0)
                            hs.append(hk)
                        for (kc, p0, p1, e) in SEGMENTS:
                            nc.vector.tensor_tensor(
                                hs[kc][p0:p1, :], hs[kc][p0:p1, :],
                                Gs[e][p0:p1, :], ALU.mult)
                        ymt = ypool.tile([C, MT], F32, tag="ymt")
                        for i in range(3):
                            psY = psY_pool.tile([C, 512], F32, tag="psY")
                            for kc in range(NCH):
                                nc.tensor.matmul(
                                    psY[:],
                                    w2T[:, kc, :],
                                    hs[kc][:, i * 512:(i + 1) * 512],
                                    start=(kc == 0), stop=False)
                            nc.tensor.matmul(
                                psY[:], b2t[:],
                                g_sb[:, goff + i * 512: goff + (i + 1) * 512],
                                start=False, stop=True)
                            nc.vector.tensor_add(
                                ymt[:, i * 512:(i + 1) * 512], psY[:],
                                xs[b][:, off + i * 512: off + (i + 1) * 512])
                        nc.sync.dma_start(out=y_d[b][:, off:off + MT], in_=ymt[:])

            # ---- misc outputs (importance / load) ------------------------
            with tc.tile_pool(name="psM", bufs=1, space="PSUM") as psM_pool:
                psM = psM_pool.tile([1, 2, E], F32, tag="psM")
                nc.tensor.matmul(psM[:, 0, :], ones[:], accimp[:],
                                 start=True, stop=True)
                nc.tensor.matmul(psM[:, 1, :], ones[:], accld[:],
                                 start=True, stop=True)
                miscs = persist.tile([1, 8], F32, tag="miscs")
                nc.vector.tensor_copy(miscs[:], psM[:].rearrange("a b c -> a (b c)"))
                nc.sync.dma_start(out=misc_d[:], in_=miscs[:])

    _split_multi_waits(nc)
    return nc


_NC_CACHE = {}


def _get_nc():
    if "nc" not in _NC_CACHE:
        _NC_CACHE["nc"] = build_program()
    return _NC_CACHE["nc"]


def _prep_shared(prompt, w_lin, b_lin, w_conv, w_gate, w1, b1, w2, b2):
    f32 = np.float32
    out = {}
    out["wconvT"] = np.ascontiguousarray(w_conv.T, dtype=f32)
    out["wgate"] = np.ascontiguousarray(w_gate, dtype=f32)
    out["wlinT"] = np.ascontiguousarray(w_lin.T / np.float32(HW_N), dtype=f32)
    out["blin2"] = np.tile(b_lin.astype(f32)[None, :], (B, 1))
    out["prompt"] = np.ascontiguousarray(prompt, dtype=f32)
    # per-expert hidden padded 340 -> 352 so expert boundaries are 32-aligned
    w1Tp = np.zeros((C, HID_P), dtype=f32)
    w2fp = np.zeros((HID_P, C), dtype=f32)
    b1f = np.zeros((HID_P,), dtype=f32)
    for e in range(E):
        w1Tp[:, e * HID_E: e * HID_E + HID] = w1[e].astype(f32)
        w2fp[e * HID_E: e * HID_E + HID] = w2[e].astype(f32)
        b1f[e * HID_E: e * HID_E + HID] = b1[e].astype(f32)
    out["w1T"] = w1Tp.astype(BF16_NP)
    out["w2T"] = w2fp.astype(BF16_NP)
    out["b1T"] = np.ascontiguousarray(b1f.reshape(NCH, C).T, dtype=f32)
    out["b2"] = b2.astype(BF16_NP)
    out["identf"] = np.eye(C, dtype=f32)
    out["identb"] = np.eye(C, dtype=f32).astype(BF16_NP)
    out["ones"] = np.ones((C, 1), dtype=f32)
    return out


def kernel(x, prompt, w_lin, b_lin, w_conv, w_gate, w1, b1, w2, b2):
    x = np.asarray(x, dtype=np.float32)
    shared = _prep_shared(np.asarray(prompt), np.asarray(w_lin),
                          np.asarray(b_lin), np.asarray(w_conv),
                          np.asarray(w_gate), np.asarray(w1),
                          np.asarray(b1), np.asarray(w2), np.asarray(b2))
    nc = _get_nc()
    in_maps = []
    for i in range(N_CORES):
        slab = np.ascontiguousarray(
            x[:, :, i * HS:(i + 1) * HS, :].reshape(B, C, TB))
        m = {"x": slab}
        m.update(shared)
        in_maps.append(m)
    res = run_bass_kernel_spmd(nc, in_maps, list(range(N_CORES)))
    y = np.empty((B, C, H, W), dtype=np.float32)
    imp = np.zeros((E,), dtype=np.float32)
    ld = np.zeros((E,), dtype=np.float32)
    for i in range(N_CORES):
        r = res.results[i]
        y[:, :, i * HS:(i + 1) * HS, :] = r["y"].reshape(B, C, HS, W)
        imp += r["misc"][0, :4]
        ld += r["misc"][0, 4:]

    def cv2(v):
        v = v.astype(np.float32)
        mean = np.float32(np.mean(v, dtype=np.float32))
        var = np.float32(np.mean((v - mean) ** 2, dtype=np.float32))
        return var / (mean * mean + np.float32(1e-10))

    loss = np.float32(1e-2) * (cv2(imp) + cv2(ld))
    return y, loss
